# revision 41
# baseline (speedup 1.0000x reference)
"""Trainium2 Bass kernel for nn_EnhancementLayerMamba (L=1 Mamba enhancement layer).

The sequence length is 1, so the selective scan collapses:
    y = delta * u * (Bm . Cm) + u * D        (A_log is dead: h0 = 0)

ZERO-COLLECTIVE design: on this 8-core axon setup every firmware collective
costs 25-50us in entry/skew regardless of payload (measured), so each core
redundantly computes the full E=2048 middle (W_in, W_x, W_dt replicated in
bf16) and is sharded only over the output z-dim (W_od/W_d/W_fg/W_o columns,
W_fbo rows). Partial outputs are summed on the host.

Layout: batch (128) on partitions for every activation; weights are the
MOVING matmul operand (N=512) and activation tiles the stationary one.
LayerNorm folds into the first matmul via K=1-seeded bias rows:
    xz = x_raw @ Wi' + outer(-mu, colsum(Wi')) + outer(sd, bias_row)
    u  = silu(inv * xz_psum)                 (per-partition ACT scale)
s = Bm . Cm is fully local per batch row, so the whole tail is ONE psum:
    z = gelu( (s*delta*P) @ W_od + P @ WodD + x @ W_d + hb_d )
    out_partial = (z * film_g) @ W_o + c[:, cslice] @ W_fbo

Host-side constant folding (weight-only transforms):
    Wi'   = diag(ln_g) @ W_in with conv tap 3 folded into the xi half
    W_od  = W_out @ W_d;  WodD = diag(D) @ W_od
    W_fbo = W_f[:, N:] @ W_o;  hb_o = b_f[N:] @ W_o + b_o
    x^2 precomputed on host for the LN variance (exact, data-local op)
"""

import json

import numpy as np
import ml_dtypes
from contextlib import ExitStack

import concourse.bass as bass
import concourse.mybir as mybir
import concourse.tile as tile
import concourse.masks as masks
import concourse.bass_utils as _bass_utils
import concourse.bass2jax as _bass2jax
from concourse.bass_utils import run_bass_kernel_spmd

R = 8            # cores
B = 128          # batch (always the partition dim for activations)
STEPS = 1024
E = 2048
DTR = 512        # dt_rank
N = 512          # model states
ZS = N // R      # 64: z-shard per core
COND = 512
CS = COND // R   # 64: cond-shard per core (for the W_fbo partial)
XW = DTR + 2 * N  # 1536: full x_dbl width

F32 = mybir.dt.float32
BF16 = mybir.dt.bfloat16
AF = mybir.ActivationFunctionType
ALU = mybir.AluOpType
AX = mybir.AxisListType

BF = ml_dtypes.bfloat16


def _split_multiwaits(bir_bytes: bytes) -> bytes:
    """The walrus in this image accepts one sync-wait per instruction
    ("Too many sync wait commands", CoreV3GenImpl setupSyncWait). Tile emits
    instructions with several waits; split the extras into single-wait
    EventSemaphore instructions on the same engine, directly before."""
    j = json.loads(bir_bytes)

    def fix(obj):
        if isinstance(obj, dict):
            for k, v in obj.items():
                if k == "instructions" and isinstance(v, list):
                    new = []
                    for ins in v:
                        si = ins.get("sync_info") if isinstance(ins, dict) else None
                        waits = si.get("on_wait") if si else None
                        if waits and len(waits) > 1:
                            for i, w in enumerate(waits[:-1]):
                                new.append({
                                    "debug": ins.get("debug", 0),
                                    "engine": ins["engine"],
                                    "ins": [], "outs": [],
                                    "name": f"{ins['name']}_w{i}",
                                    "opcode": "EventSemaphore",
                                    "sync_info": {"on_update": [],
                                                  "on_wait": [w]},
                                })
                            si["on_wait"] = waits[-1:]
                        new.append(ins)
                    obj[k] = new
                else:
                    fix(v)
        elif isinstance(obj, list):
            for v in obj:
                fix(v)

    fix(j)
    return json.dumps(j).encode()


_ORIG_COMPILE_BIR = _bass_utils.compile_bir_kernel


def _patched_compile_bir_kernel(bir_json, tmpdir, neff_name="file.neff"):
    if isinstance(bir_json, str):
        bir_json = _split_multiwaits(bir_json.encode())
    else:
        bir_json = _split_multiwaits(bytes(bir_json))
    return _ORIG_COMPILE_BIR(bir_json, tmpdir, neff_name=neff_name)


if getattr(_bass_utils.compile_bir_kernel, "__name__", "") != "_patched_compile_bir_kernel":
    _bass_utils.compile_bir_kernel = _patched_compile_bir_kernel
    _bass2jax.compile_bir_kernel = _patched_compile_bir_kernel


def build_nc() -> bass.Bass:
    nc = bass.Bass(num_devices=R)

    xpk_d = nc.dram_tensor("xpk", [STEPS, 2 * B], BF16, kind="ExternalInput")
    Wi_d = nc.dram_tensor("Wi", [STEPS, 2 * E], BF16, kind="ExternalInput")
    seedR_d = nc.dram_tensor("seedR", [1, 4 * E], BF16, kind="ExternalInput")
    Wx_d = nc.dram_tensor("Wx", [E, XW], BF16, kind="ExternalInput")
    Wdt_d = nc.dram_tensor("Wdt", [DTR, E], BF16, kind="ExternalInput")
    sdt_d = nc.dram_tensor("sdt", [2, E], BF16, kind="ExternalInput")
    Wod_d = nc.dram_tensor("Wod", [E, ZS], BF16, kind="ExternalInput")
    Drow_d = nc.dram_tensor("Drow", [1, E], BF16, kind="ExternalInput")
    Wd_d = nc.dram_tensor("Wd", [STEPS, ZS], BF16, kind="ExternalInput")
    shd_d = nc.dram_tensor("shd", [2, ZS], BF16, kind="ExternalInput")
    cT_d = nc.dram_tensor("cT", [COND, B], BF16, kind="ExternalInput")
    Wfg_d = nc.dram_tensor("Wfg", [COND, ZS], BF16, kind="ExternalInput")
    sfg_d = nc.dram_tensor("sfg", [2, ZS], BF16, kind="ExternalInput")
    Wo_d = nc.dram_tensor("Wo", [ZS, STEPS], BF16, kind="ExternalInput")
    Wfbo_d = nc.dram_tensor("Wfbo", [CS, STEPS], BF16, kind="ExternalInput")
    cTs_d = nc.dram_tensor("cTs", [CS, B], BF16, kind="ExternalInput")

    out_d = nc.dram_tensor("outp", [B, STEPS], BF16, kind="ExternalOutput")

    with ExitStack() as ctx:
        tc = ctx.enter_context(tile.TileContext(nc))
        wp = ctx.enter_context(tc.tile_pool(name="w", bufs=1))
        ap = ctx.enter_context(tc.tile_pool(name="a", bufs=1))
        apc = ctx.enter_context(tc.tile_pool(name="ac", bufs=2))
        apd = ctx.enter_context(tc.tile_pool(name="ad", bufs=1))
        pmm = ctx.enter_context(tc.tile_pool(name="pmm", bufs=2, space="PSUM"))
        ptr = ctx.enter_context(tc.tile_pool(name="ptr", bufs=2, space="PSUM"))
        pax = ctx.enter_context(tc.tile_pool(name="pax", bufs=1, space="PSUM"))

        # ---- constants ----
        ident_b = wp.tile([128, 128], BF16, name="ident_b", tag="ident_b")
        masks.make_identity(nc, ident_b[:])
        ones_c = wp.tile([128, 1], BF16, name="ones_c", tag="ones_c")
        nc.vector.memset(ones_c[:], 1.0)
        ones2 = wp.tile([2, B], BF16, name="ones2", tag="ones2")
        nc.vector.memset(ones2[:], 1.0)
        # prefetch the silu table (ACT table cache holds ~2 sets)
        dmy = ap.tile([1, 2], F32, name="dmy", tag="dmy")
        nc.vector.memset(dmy[:, 0:1], 1.0)
        nc.scalar.activation(dmy[:, 1:2], dmy[:, 0:1], AF.Silu)

        # ---- input DMAs (critical order: xpk, Wi first) ----
        xpk = ap.tile([128, 8 * 2 * B], BF16, name="xpk", tag="xpk")
        nc.sync.dma_start(xpk[:].rearrange("p (k f) -> p k f", f=2 * B),
                          xpk_d.rearrange("(k p) f -> p k f", p=128))
        wi = wp.tile([128, 8 * 2 * E], BF16, name="wi", tag="wi")
        for k in range(8):
            nc.sync.dma_start(wi[:, 2 * E * k:2 * E * k + E],
                              Wi_d[128 * k:128 * (k + 1), 0:E])
        seedR = wp.tile([1, 4 * E], BF16, name="seedR", tag="seedR")
        nc.scalar.dma_start(seedR[:], seedR_d[:, :])
        wx = wp.tile([128, 16 * XW], BF16, name="wx", tag="wx")
        for k in range(8):
            nc.sync.dma_start(
                wx[:, XW * 2 * k:XW * 2 * (k + 1)].rearrange(
                    "p (t m) -> p t m", m=XW),
                Wx_d[256 * k:256 * (k + 1), :].rearrange(
                    "(t p) m -> p t m", p=128))
        # res half of Wi streams AFTER Wx: gate/P are only needed at the end
        for k in range(8):
            nc.sync.dma_start(wi[:, 2 * E * k + E:2 * E * (k + 1)],
                              Wi_d[128 * k:128 * (k + 1), E:2 * E])
        wdt = wp.tile([128, 4 * E], BF16, name="wdt", tag="wdt")
        for k in range(4):
            nc.scalar.dma_start(wdt[:, E * k:E * (k + 1)],
                                Wdt_d[128 * k:128 * (k + 1), :])
        drow = wp.tile([1, E], BF16, name="drow", tag="drow")
        nc.scalar.dma_start(drow[:], Drow_d[:, :])
        sdt = wp.tile([2, E], BF16, name="sdt", tag="sdt")
        nc.scalar.dma_start(sdt[:], sdt_d[:, :])
        cpk = ap.tile([128, 4 * B], BF16, name="cpk", tag="cpk")
        nc.scalar.dma_start(cpk[:].rearrange("p (k b) -> p k b", b=B),
                            cT_d.rearrange("(k p) b -> p k b", p=128))
        wfg = wp.tile([128, 4 * ZS], BF16, name="wfg", tag="wfg")
        nc.scalar.dma_start(wfg[:].rearrange("p (k m) -> p k m", m=ZS),
                            Wfg_d.rearrange("(k p) m -> p k m", p=128))
        sfg = wp.tile([2, ZS], BF16, name="sfg", tag="sfg")
        nc.scalar.dma_start(sfg[:], sfg_d[:, :])
        shd = wp.tile([2, ZS], BF16, name="shd", tag="shd")
        nc.scalar.dma_start(shd[:], shd_d[:, :])
        wod = wp.tile([128, 16 * ZS], BF16, name="wod", tag="wod")
        nc.gpsimd.dma_start(wod[:].rearrange("p (k m) -> p k m", m=ZS),
                            Wod_d.rearrange("(k p) m -> p k m", p=128))
        wd = wp.tile([128, 8 * ZS], BF16, name="wd", tag="wd")
        nc.gpsimd.dma_start(wd[:].rearrange("p (k m) -> p k m", m=ZS),
                            Wd_d.rearrange("(k p) m -> p k m", p=128))
        wo = wp.tile([ZS, STEPS], BF16, name="wo", tag="wo")
        nc.gpsimd.dma_start(wo[:], Wo_d[:, :])
        wfbo = wp.tile([CS, STEPS], BF16, name="wfbo", tag="wfbo")
        nc.gpsimd.dma_start(wfbo[:], Wfbo_d[:, :])
        cts = ap.tile([CS, B], BF16, name="cts", tag="cts")
        nc.gpsimd.dma_start(cts[:], cTs_d[:, :])

        # ---- D broadcast [b, E] via K=1 ones matmuls (no data deps) ----
        dbar = ap.tile([128, E], BF16, name="dbar", tag="dbar")
        for j in range(4):
            psDB = pmm.tile([128, 512], F32, name=f"psDB{j}", tag="mm")
            nc.tensor.matmul(psDB[:], ones2[0:1, :],
                             drow[:, 512 * j:512 * (j + 1)],
                             start=True, stop=True)
            if j % 2 == 0:
                nc.vector.tensor_copy(dbar[:, 512 * j:512 * (j + 1)], psDB[:])
            else:
                nc.scalar.copy(dbar[:, 512 * j:512 * (j + 1)], psDB[:])

        # ---- LN stats (psum via rotating mm pool; freed before xz1) ----
        psS = pmm.tile([1, 2 * B], F32, name="psS", tag="mm")
        for k in range(8):
            nc.tensor.matmul(psS[:], ones_c[:, :], xpk[:, 2 * B * k:2 * B * (k + 1)],
                             start=(k == 0), stop=(k == 7))
        mu_r = ap.tile([1, B], F32, name="mu_r", tag="mu_r")
        nc.vector.tensor_scalar_mul(mu_r[:], psS[:, 0:B], 1.0 / STEPS)
        ex2 = ap.tile([1, B], F32, name="ex2", tag="ex2")
        nc.vector.tensor_scalar_mul(ex2[:], psS[:, B:2 * B], 1.0 / STEPS)
        m2 = ap.tile([1, B], F32, name="m2", tag="m2")
        nc.vector.tensor_mul(m2[:], mu_r[:], mu_r[:])
        var = ap.tile([1, B], F32, name="var", tag="var")
        nc.vector.tensor_sub(var[:], ex2[:], m2[:])
        vpe = ap.tile([1, B], F32, name="vpe", tag="vpe")
        nc.vector.tensor_scalar_add(vpe[:], var[:], 1e-5)
        lnv = ap.tile([1, B], F32, name="lnv", tag="lnv")
        nc.scalar.activation(lnv[:], vpe[:], AF.Ln)
        inv_r = ap.tile([1, B], F32, name="inv_r", tag="inv_r")
        nc.scalar.activation(inv_r[:], lnv[:], AF.Exp, scale=-0.5)
        sd_r = ap.tile([1, B], F32, name="sd_r", tag="sd_r")
        nc.vector.tensor_mul(sd_r[:], vpe[:], inv_r[:])
        nmu_r = ap.tile([1, B], F32, name="nmu_r", tag="nmu_r")
        nc.vector.tensor_scalar_mul(nmu_r[:], mu_r[:], -1.0)
        nmu_b = ap.tile([1, B], BF16, name="nmu_b", tag="nmu_b")
        nc.vector.tensor_copy(nmu_b[:], nmu_r[:])
        sd_b = ap.tile([1, B], BF16, name="sd_b", tag="sd_b")
        nc.vector.tensor_copy(sd_b[:], sd_r[:])

        # invT [b, 1] f32 silu scale, via bf16 hi/lo PE transposes
        ivh = ap.tile([1, B], BF16, name="ivh", tag="ivh")
        nc.vector.tensor_copy(ivh[:], inv_r[:])
        ivl = ap.tile([1, B], BF16, name="ivl", tag="ivl")
        nc.vector.tensor_sub(ivl[:], inv_r[:], ivh[:])
        psITh = ptr.tile([128, 1], BF16, name="psITh", tag="tr")
        nc.tensor.transpose(psITh[:], ivh[:], ident_b[0:1, 0:1])
        psITl = ptr.tile([128, 1], BF16, name="psITl", tag="tr")
        nc.tensor.transpose(psITl[:], ivl[:], ident_b[0:1, 0:1])
        ivhc = ap.tile([128, 1], BF16, name="ivhc", tag="ivhc")
        nc.vector.tensor_copy(ivhc[:], psITh[:])
        invT = ap.tile([128, 1], F32, name="invT", tag="invT")
        nc.vector.tensor_add(invT[:], ivhc[:], psITl[:])

        # ---- stage 1: full xz in 4 chunks of 1024 cols; u, gate, P ----
        # chunk c covers Wi' cols [1024c, 1024c+1024): c=0,1 xi; c=2,3 res.
        # order (0, 2, 1, 3) so each xi/res pair multiplies into P and frees.
        Pb = ap.tile([128, E], BF16, name="Pb", tag="Pb")
        ub = ap.tile([128, E], BF16, name="ub", tag="ub")
        uT_sb = ap.tile([128, 16 * B], BF16, name="uT_sb", tag="stA")
        acts = {}
        for c in (0, 1, 2, 3):
            psXZ = pmm.tile([128, 1024], F32, name=f"psXZ{c}", tag="mm")
            for h in range(2):
                lo = 1024 * c + 512 * h
                nc.tensor.matmul(psXZ[:, 512 * h:512 * (h + 1)], nmu_b[:],
                                 seedR[:, lo:lo + 512], start=True, stop=False)
                nc.tensor.matmul(psXZ[:, 512 * h:512 * (h + 1)], sd_b[:],
                                 seedR[:, 4096 + lo:4096 + lo + 512],
                                 start=False, stop=False)
                for k in range(8):
                    nc.tensor.matmul(
                        psXZ[:, 512 * h:512 * (h + 1)],
                        xpk[:, 2 * B * k:2 * B * k + B],
                        wi[:, 2 * E * k + lo:2 * E * k + lo + 512],
                        start=False, stop=(k == 7))
            act = apc.tile([128, 1024], BF16, name=f"act{c}", tag="uact")
            nc.scalar.activation(act[:], psXZ[:], AF.Silu, scale=invT[:, 0:1])
            acts[c] = act
            if c < 2:
                nc.vector.tensor_copy(ub[:, 1024 * c:1024 * (c + 1)], act[:])
                for j in range(8):
                    psT = ptr.tile([128, 128], BF16, name=f"psU{c}_{j}", tag="tr")
                    nc.tensor.transpose(psT[:], act[:, 128 * j:128 * (j + 1)],
                                        ident_b[:, :])
                    dst = uT_sb[:, 1024 * c + 128 * j:1024 * c + 128 * (j + 1)]
                    if j % 2 == 0:
                        nc.vector.tensor_copy(dst, psT[:])
                    else:
                        nc.scalar.copy(dst, psT[:])
            else:
                nc.vector.tensor_mul(Pb[:, 1024 * (c - 2):1024 * (c - 1)],
                                     ub[:, 1024 * (c - 2):1024 * (c - 1)], act[:])

        # ---- x_dbl: d_r [b,512] and Bm|Cm [b,1024], full ----
        psDR = pmm.tile([128, DTR], F32, name="psDR", tag="mm")
        for k in range(16):
            nc.tensor.matmul(psDR[:], uT_sb[:, B * k:B * (k + 1)],
                             wx[:, XW * k:XW * k + DTR],
                             start=(k == 0), stop=(k == 15))
        psBC = pmm.tile([128, 2 * N], F32, name="psBC", tag="mm")
        for k in range(16):
            for h in range(2):
                nc.tensor.matmul(psBC[:, 512 * h:512 * (h + 1)],
                                 uT_sb[:, B * k:B * (k + 1)],
                                 wx[:, XW * k + DTR + 512 * h:
                                    XW * k + DTR + 512 * (h + 1)],
                                 start=(k == 0), stop=(k == 15))
        drb = ap.tile([128, DTR], BF16, name="drb", tag="drb")
        nc.vector.tensor_copy(drb[:], psDR[:])
        bc_sb = ap.tile([128, 2 * N], BF16, name="bc_sb", tag="bc_sb")
        nc.scalar.copy(bc_sb[:], psBC[:])
        sprod = ap.tile([128, N], BF16, name="sprod", tag="sprod")
        nc.vector.tensor_mul(sprod[:], bc_sb[:, 0:N], bc_sb[:, N:2 * N])
        s_p = ap.tile([128, 1], F32, name="s_p", tag="s_p")
        nc.vector.tensor_reduce(s_p[:], sprod[:], AX.X, ALU.add)

        # d_r transposes -> [dr-tile, b] stationaries
        drT = ap.tile([128, 4 * B], BF16, name="drT", tag="drT")
        for j in range(4):
            psT = ptr.tile([128, 128], BF16, name=f"psD{j}", tag="tr")
            nc.tensor.transpose(psT[:], drb[:, 128 * j:128 * (j + 1)], ident_b[:, :])
            if j % 2 == 0:
                nc.vector.tensor_copy(drT[:, 128 * j:128 * (j + 1)], psT[:])
            else:
                nc.scalar.copy(drT[:, 128 * j:128 * (j + 1)], psT[:])

        # ---- delta (full, 2 chunks); y1s = s*delta*P ; Pb ----
        y1b = ap.tile([128, E], BF16, name="y1b", tag="y1b")
        for c in range(2):
            psD = pmm.tile([128, 1024], F32, name=f"psDl{c}", tag="mm")
            for h in range(2):
                lo = 1024 * c + 512 * h
                nc.tensor.matmul(psD[:, 512 * h:512 * (h + 1)], ones2[:, :],
                                 sdt[:, lo:lo + 512], start=True, stop=False)
                for j in range(4):
                    nc.tensor.matmul(
                        psD[:, 512 * h:512 * (h + 1)],
                        drT[:, B * j:B * (j + 1)],
                        wdt[:, E * j + lo:E * j + lo + 512],
                        start=False, stop=(j == 3))
            exd = apd.tile([128, 1024], BF16, name=f"exd{c}", tag="dact")
            nc.scalar.activation(exd[:], psD[:], AF.Exp)
            dl = apd.tile([128, 1024], BF16, name=f"dl{c}", tag="dact2")
            nc.scalar.activation(dl[:], exd[:], AF.Ln, bias=1.0)
            ds = apd.tile([128, 1024], BF16, name=f"ds{c}", tag="dact")
            nc.vector.tensor_scalar_mul(ds[:], dl[:], s_p[:, 0:1])
            ds2 = apd.tile([128, 1024], BF16, name=f"ds2{c}", tag="dact2")
            nc.vector.tensor_add(ds2[:], ds[:], dbar[:, 1024 * c:1024 * (c + 1)])
            nc.vector.tensor_mul(y1b[:, 1024 * c:1024 * (c + 1)], ds2[:],
                                 Pb[:, 1024 * c:1024 * (c + 1)])

        # ---- tail psum: hb_d seed + x @ Wd + FiLM gain (emitted late so the
        # PE never stalls on the late-arriving Wd/Wfg weights) ----
        psQ = pax.tile([128, ZS], F32, name="psQ", tag="paxq")
        nc.tensor.matmul(psQ[:], ones2[:, :], shd[:, :], start=True, stop=False)
        for k in range(8):
            nc.tensor.matmul(psQ[:], xpk[:, 2 * B * k:2 * B * k + B],
                             wd[:, ZS * k:ZS * (k + 1)], start=False, stop=False)
        psG = pax.tile([128, ZS], F32, name="psG", tag="paxg")
        nc.tensor.matmul(psG[:], ones2[:, :], sfg[:, :], start=True, stop=False)
        for k in range(4):
            nc.tensor.matmul(psG[:], cpk[:, B * k:B * (k + 1)],
                             wfg[:, ZS * k:ZS * (k + 1)], start=False, stop=(k == 3))
        g_sb = ap.tile([128, ZS], F32, name="g_sb", tag="g_sb")
        nc.vector.tensor_copy(g_sb[:], psG[:])

        # transposes of y1s and P -> stationaries for the q contraction
        y1T = ap.tile([128, 16 * B], BF16, name="y1T", tag="stA")
        for j in range(16):
            psT = ptr.tile([128, 128], BF16, name=f"psY{j}", tag="tr")
            nc.tensor.transpose(psT[:], y1b[:, 128 * j:128 * (j + 1)], ident_b[:, :])
            if j % 2 == 0:
                nc.vector.tensor_copy(y1T[:, 128 * j:128 * (j + 1)], psT[:])
            else:
                nc.scalar.copy(y1T[:, 128 * j:128 * (j + 1)], psT[:])

        # ---- finish the tail psum: + y_comb @ Wod ; gelu ; out ----
        for k in range(16):
            nc.tensor.matmul(psQ[:], y1T[:, B * k:B * (k + 1)],
                             wod[:, ZS * k:ZS * (k + 1)],
                             start=False, stop=(k == 15))
        z = ap.tile([128, ZS], F32, name="z", tag="z")
        nc.scalar.activation(z[:], psQ[:], AF.Gelu)
        zf = ap.tile([128, ZS], BF16, name="zf", tag="zf")
        nc.vector.tensor_mul(zf[:], z[:], g_sb[:])
        psZT = ptr.tile([ZS, 128], BF16, name="psZT", tag="tr")
        nc.tensor.transpose(psZT[:], zf[:], ident_b[:, :])
        zfT = ap.tile([ZS, B], BF16, name="zfT", tag="zfT")
        nc.vector.tensor_copy(zfT[:], psZT[:])

        out_sb = ap.tile([128, STEPS], BF16, name="out_sb", tag="out_sb")
        for m in range(2):
            psO = pmm.tile([128, 512], F32, name=f"psO{m}", tag="mm")
            nc.tensor.matmul(psO[:], zfT[:, :], wo[:, 512 * m:512 * (m + 1)],
                             start=True, stop=False)
            nc.tensor.matmul(psO[:], cts[:, :], wfbo[:, 512 * m:512 * (m + 1)],
                             start=False, stop=True)
            if m == 0:
                nc.vector.tensor_copy(out_sb[:, 512 * m:512 * (m + 1)], psO[:])
            else:
                nc.scalar.copy(out_sb[:, 512 * m:512 * (m + 1)], psO[:])
            nc.sync.dma_start(out_d[:, 512 * m:512 * (m + 1)],
                              out_sb[:, 512 * m:512 * (m + 1)])

    return nc


_CACHE = {}


def _get_nc() -> bass.Bass:
    if "nc" not in _CACHE:
        _CACHE["nc"] = build_nc()
    return _CACHE["nc"]


def _hilo(a):
    hi = a.astype(BF)
    lo = (a - hi.astype(np.float32)).astype(BF)
    return hi, lo


def kernel(**inputs) -> np.ndarray:
    inp = {k: np.asarray(v) for k, v in inputs.items()}
    f32 = np.float32
    x = inp["x"].reshape(B, STEPS).astype(f32)
    c = inp["c"].astype(f32)
    ln_g = inp["ln_g"].astype(f32)
    ln_b = inp["ln_b"].astype(f32)
    W_in = inp["W_in"].astype(f32)
    conv_w = inp["conv_w"].astype(f32)
    conv_b = inp["conv_b"].astype(f32)
    W_x = inp["W_x"].astype(f32)
    W_dt = inp["W_dt"].astype(f32)
    b_dt = inp["b_dt"].astype(f32)
    D = inp["D"].astype(f32)
    W_out = inp["W_out"].astype(f32)
    b_out = inp["b_out"].astype(f32)
    W_d = inp["W_d"].astype(f32)
    b_d = inp["b_d"].astype(f32)
    W_f = inp["W_f"].astype(f32)
    b_f = inp["b_f"].astype(f32)
    W_o = inp["W_o"].astype(f32)
    b_o = inp["b_o"].astype(f32)

    # host constant folding (weight-only)
    cw = conv_w[3, 0, :]
    Wi_full = np.concatenate([(ln_g[:, None] * W_in[:, :E]) * cw[None, :],
                              ln_g[:, None] * W_in[:, E:]], axis=1)  # (1024,4096)
    bias_xz = ln_b @ W_in
    bias_k = np.concatenate([bias_xz[:E] * cw + conv_b, bias_xz[E:]])  # (4096,)
    W_od = W_out @ W_d                      # (2048, 512)
    hb_d = b_out @ W_d + b_d                # (512,)
    W_fg = W_f[:, :N]
    b_fg = b_f[:N]
    W_fbo = W_f[:, N:] @ W_o                # (512, 1024)
    hb_o = b_f[N:] @ W_o + b_o              # (1024,)

    Wi_bf = Wi_full.astype(BF)
    cs_full = Wi_bf.astype(f32).sum(axis=0)                    # colsum of bf16 Wi
    seedR = np.concatenate([cs_full, bias_k])[None, :].astype(BF)  # (1, 8192)
    dh, dl = _hilo(b_dt)
    sdt = np.stack([dh, dl]).astype(BF)                        # (2, 2048)

    xT_bf = np.ascontiguousarray(x.T).astype(BF)               # (1024, 128)
    xsqT_bf = np.ascontiguousarray((x * x).T).astype(BF)
    xpk = np.concatenate([xT_bf, xsqT_bf], axis=1)             # (1024, 256)
    cT_bf = np.ascontiguousarray(c.T).astype(BF)               # (512, 128)
    Wx_bf = np.ascontiguousarray(W_x).astype(BF)               # (2048, 1536)
    Wdt_bf = np.ascontiguousarray(W_dt).astype(BF)             # (512, 2048)

    in_maps = []
    for k in range(R):
        zs = slice(ZS * k, ZS * (k + 1))
        cs = slice(CS * k, CS * (k + 1))
        hh, hl = _hilo(hb_d[zs])
        gh, gl = _hilo(b_fg[zs])
        in_maps.append({
            "xpk": xpk,
            "Wi": Wi_bf,
            "seedR": seedR,
            "Wx": Wx_bf,
            "Wdt": Wdt_bf,
            "sdt": sdt,
            "Wod": np.ascontiguousarray(W_od[:, zs]).astype(BF),
            "Drow": D[None, :].astype(BF),
            "Wd": np.ascontiguousarray(W_d[:, zs]).astype(BF),
            "shd": np.stack([hh, hl]).astype(BF),
            "cT": cT_bf,
            "Wfg": np.ascontiguousarray(W_fg[:, zs]).astype(BF),
            "sfg": np.stack([gh, gl]).astype(BF),
            "Wo": np.ascontiguousarray(W_o[zs, :]).astype(BF),
            "Wfbo": np.ascontiguousarray(W_fbo[cs, :]).astype(BF),
            "cTs": np.ascontiguousarray(cT_bf[cs, :]),
        })

    nc = _get_nc()
    res = run_bass_kernel_spmd(nc, in_maps, core_ids=list(range(R)),
                               **_CACHE.get("run_kwargs", {}))
    _CACHE["last_results"] = res
    out = np.zeros((B, STEPS), np.float64)
    for r in res.results:
        out += r["outp"].astype(np.float64)
    out = out.astype(f32) + hb_o[None, :]
    return out.astype(f32)


# revision 43
# speedup vs baseline: 1.1693x; 1.1693x over previous
"""Trainium2 Bass kernel for nn_EnhancementLayerMamba (L=1 Mamba enhancement layer).

The sequence length is 1, so the selective scan collapses:
    y = delta * u * (Bm . Cm) + u * D        (A_log is dead: h0 = 0)

ZERO-COLLECTIVE design: on this 8-core axon setup every firmware collective
costs 25-50us in entry/skew regardless of payload (measured), so each core
redundantly computes the full E=2048 middle (W_in, W_x, W_dt replicated in
bf16) and is sharded only over the output z-dim (W_od/W_d/W_fg/W_o columns,
W_fbo rows). Partial outputs are summed on the host.

Layout: batch (128) on partitions for every activation; weights are the
MOVING matmul operand (N=512) and activation tiles the stationary one.
LayerNorm folds into the first matmul via K=1-seeded bias rows:
    xz = x_raw @ Wi' + outer(-mu, colsum(Wi')) + outer(sd, bias_row)
    u  = silu(inv * xz_psum)                 (per-partition ACT scale)
s = Bm . Cm is fully local per batch row, so the whole tail is ONE psum:
    z = gelu( (s*delta*P) @ W_od + P @ WodD + x @ W_d + hb_d )
    out_partial = (z * film_g) @ W_o + c[:, cslice] @ W_fbo

Host-side constant folding (weight-only transforms):
    Wi'   = diag(ln_g) @ W_in with conv tap 3 folded into the xi half
    W_od  = W_out @ W_d;  WodD = diag(D) @ W_od
    W_fbo = W_f[:, N:] @ W_o;  hb_o = b_f[N:] @ W_o + b_o
    x^2 precomputed on host for the LN variance (exact, data-local op)
"""

import json

import numpy as np
import ml_dtypes
from contextlib import ExitStack

import concourse.bass as bass
import concourse.mybir as mybir
import concourse.tile as tile
import concourse.masks as masks
import concourse.bass_utils as _bass_utils
import concourse.bass2jax as _bass2jax
from concourse.bass_utils import run_bass_kernel_spmd

R = 8            # cores
B = 128          # batch (always the partition dim for activations)
STEPS = 1024
E = 2048
DTR = 512        # dt_rank
N = 512          # model states
ZS = N // R      # 64: z-shard per core
COND = 512
CS = COND // R   # 64: cond-shard per core (for the W_fbo partial)
XW = DTR + 2 * N  # 1536: full x_dbl width

F32 = mybir.dt.float32
BF16 = mybir.dt.bfloat16
AF = mybir.ActivationFunctionType
ALU = mybir.AluOpType
AX = mybir.AxisListType

BF = ml_dtypes.bfloat16


def _split_multiwaits(bir_bytes: bytes) -> bytes:
    """The walrus in this image accepts one sync-wait per instruction
    ("Too many sync wait commands", CoreV3GenImpl setupSyncWait). Tile emits
    instructions with several waits; split the extras into single-wait
    EventSemaphore instructions on the same engine, directly before."""
    j = json.loads(bir_bytes)

    def fix(obj):
        if isinstance(obj, dict):
            for k, v in obj.items():
                if k == "instructions" and isinstance(v, list):
                    new = []
                    for ins in v:
                        si = ins.get("sync_info") if isinstance(ins, dict) else None
                        waits = si.get("on_wait") if si else None
                        if waits and len(waits) > 1:
                            for i, w in enumerate(waits[:-1]):
                                new.append({
                                    "debug": ins.get("debug", 0),
                                    "engine": ins["engine"],
                                    "ins": [], "outs": [],
                                    "name": f"{ins['name']}_w{i}",
                                    "opcode": "EventSemaphore",
                                    "sync_info": {"on_update": [],
                                                  "on_wait": [w]},
                                })
                            si["on_wait"] = waits[-1:]
                        new.append(ins)
                    obj[k] = new
                else:
                    fix(v)
        elif isinstance(obj, list):
            for v in obj:
                fix(v)

    fix(j)
    return json.dumps(j).encode()


_ORIG_COMPILE_BIR = _bass_utils.compile_bir_kernel


def _patched_compile_bir_kernel(bir_json, tmpdir, neff_name="file.neff"):
    if isinstance(bir_json, str):
        bir_json = _split_multiwaits(bir_json.encode())
    else:
        bir_json = _split_multiwaits(bytes(bir_json))
    return _ORIG_COMPILE_BIR(bir_json, tmpdir, neff_name=neff_name)


if getattr(_bass_utils.compile_bir_kernel, "__name__", "") != "_patched_compile_bir_kernel":
    _bass_utils.compile_bir_kernel = _patched_compile_bir_kernel
    _bass2jax.compile_bir_kernel = _patched_compile_bir_kernel


def build_nc() -> bass.Bass:
    nc = bass.Bass(num_devices=R)

    xpk_d = nc.dram_tensor("xpk", [128, 16 * B], BF16, kind="ExternalInput")
    Wi_d = nc.dram_tensor("Wi", [STEPS, 2 * E], BF16, kind="ExternalInput")
    seedR_d = nc.dram_tensor("seedR", [1, 4 * E], BF16, kind="ExternalInput")
    Wx_d = nc.dram_tensor("Wx", [E, XW], BF16, kind="ExternalInput")
    Wdt_d = nc.dram_tensor("Wdt", [128, 4 * E], BF16, kind="ExternalInput")
    sdt_d = nc.dram_tensor("sdt", [2, E], BF16, kind="ExternalInput")
    Wod_d = nc.dram_tensor("Wod", [128, 16 * ZS], BF16, kind="ExternalInput")
    Drow_d = nc.dram_tensor("Drow", [1, E], BF16, kind="ExternalInput")
    Wd_d = nc.dram_tensor("Wd", [128, 8 * ZS], BF16, kind="ExternalInput")
    shd_d = nc.dram_tensor("shd", [2, ZS], BF16, kind="ExternalInput")
    cT_d = nc.dram_tensor("cT", [128, 4 * B], BF16, kind="ExternalInput")
    Wfg_d = nc.dram_tensor("Wfg", [128, 4 * ZS], BF16, kind="ExternalInput")
    sfg_d = nc.dram_tensor("sfg", [2, ZS], BF16, kind="ExternalInput")
    Wo_d = nc.dram_tensor("Wo", [ZS, STEPS], BF16, kind="ExternalInput")
    Wfbo_d = nc.dram_tensor("Wfbo", [CS, STEPS], BF16, kind="ExternalInput")
    cTs_d = nc.dram_tensor("cTs", [CS, B], BF16, kind="ExternalInput")

    out_d = nc.dram_tensor("outp", [B, STEPS], BF16, kind="ExternalOutput")

    with ExitStack() as ctx:
        tc = ctx.enter_context(tile.TileContext(nc))
        wp = ctx.enter_context(tc.tile_pool(name="w", bufs=1))
        ap = ctx.enter_context(tc.tile_pool(name="a", bufs=1))
        apc = ctx.enter_context(tc.tile_pool(name="ac", bufs=2))
        apd = ctx.enter_context(tc.tile_pool(name="ad", bufs=1))
        pmm = ctx.enter_context(tc.tile_pool(name="pmm", bufs=2, space="PSUM"))
        ptr = ctx.enter_context(tc.tile_pool(name="ptr", bufs=2, space="PSUM"))
        pax = ctx.enter_context(tc.tile_pool(name="pax", bufs=1, space="PSUM"))

        # ---- constants ----
        ident_b = wp.tile([128, 128], BF16, name="ident_b", tag="ident_b")
        masks.make_identity(nc, ident_b[:])
        ones_c = wp.tile([128, 1], BF16, name="ones_c", tag="ones_c")
        nc.vector.memset(ones_c[:], 1.0)
        ones2 = wp.tile([2, B], BF16, name="ones2", tag="ones2")
        nc.vector.memset(ones2[:], 1.0)
        # prefetch the silu table (ACT table cache holds ~2 sets)
        dmy = ap.tile([1, 2], F32, name="dmy", tag="dmy")
        nc.vector.memset(dmy[:, 0:1], 1.0)
        nc.scalar.activation(dmy[:, 1:2], dmy[:, 0:1], AF.Silu)

        # ---- input DMAs (critical order: xpk, Wi first) ----
        xpk = ap.tile([128, 8 * 2 * B], BF16, name="xpk", tag="xpk")
        nc.sync.dma_start(xpk[:], xpk_d[:, :])
        wi = wp.tile([128, 8 * 2 * E], BF16, name="wi", tag="wi")
        for k in range(4):
            nc.sync.dma_start(
                wi[:, 2 * E * 2 * k:2 * E * 2 * (k + 1)].rearrange(
                    "p (t m) -> p t m", m=2 * E),
                Wi_d[256 * k:256 * (k + 1), :].rearrange(
                    "(t p) m -> p t m", p=128))
        seedR = wp.tile([1, 4 * E], BF16, name="seedR", tag="seedR")
        nc.scalar.dma_start(seedR[:], seedR_d[:, :])
        wx = wp.tile([128, 16 * XW], BF16, name="wx", tag="wx")
        for k in range(8):
            nc.sync.dma_start(
                wx[:, XW * 2 * k:XW * 2 * (k + 1)].rearrange(
                    "p (t m) -> p t m", m=XW),
                Wx_d[256 * k:256 * (k + 1), :].rearrange(
                    "(t p) m -> p t m", p=128))
        wdt = wp.tile([128, 4 * E], BF16, name="wdt", tag="wdt")
        nc.scalar.dma_start(wdt[:], Wdt_d[:, :])
        drow = wp.tile([1, E], BF16, name="drow", tag="drow")
        nc.scalar.dma_start(drow[:], Drow_d[:, :])
        sdt = wp.tile([2, E], BF16, name="sdt", tag="sdt")
        nc.scalar.dma_start(sdt[:], sdt_d[:, :])
        cpk = ap.tile([128, 4 * B], BF16, name="cpk", tag="cpk")
        nc.scalar.dma_start(cpk[:], cT_d[:, :])
        wfg = wp.tile([128, 4 * ZS], BF16, name="wfg", tag="wfg")
        nc.scalar.dma_start(wfg[:], Wfg_d[:, :])
        sfg = wp.tile([2, ZS], BF16, name="sfg", tag="sfg")
        nc.scalar.dma_start(sfg[:], sfg_d[:, :])
        shd = wp.tile([2, ZS], BF16, name="shd", tag="shd")
        nc.scalar.dma_start(shd[:], shd_d[:, :])
        wod = wp.tile([128, 16 * ZS], BF16, name="wod", tag="wod")
        nc.gpsimd.dma_start(wod[:], Wod_d[:, :])
        wd = wp.tile([128, 8 * ZS], BF16, name="wd", tag="wd")
        nc.gpsimd.dma_start(wd[:], Wd_d[:, :])
        wo = wp.tile([ZS, STEPS], BF16, name="wo", tag="wo")
        nc.gpsimd.dma_start(wo[:], Wo_d[:, :])
        wfbo = wp.tile([CS, STEPS], BF16, name="wfbo", tag="wfbo")
        nc.gpsimd.dma_start(wfbo[:], Wfbo_d[:, :])
        cts = ap.tile([CS, B], BF16, name="cts", tag="cts")
        nc.gpsimd.dma_start(cts[:], cTs_d[:, :])

        # ---- D broadcast [b, E] via K=1 ones matmuls (no data deps) ----
        dbar = ap.tile([128, E], BF16, name="dbar", tag="dbar")
        for j in range(4):
            psDB = pmm.tile([128, 512], F32, name=f"psDB{j}", tag="mm")
            nc.tensor.matmul(psDB[:], ones2[0:1, :],
                             drow[:, 512 * j:512 * (j + 1)],
                             start=True, stop=True)
            if j % 2 == 0:
                nc.vector.tensor_copy(dbar[:, 512 * j:512 * (j + 1)], psDB[:])
            else:
                nc.scalar.copy(dbar[:, 512 * j:512 * (j + 1)], psDB[:])

        # ---- LN stats (psum via rotating mm pool; freed before xz1) ----
        psS = pmm.tile([1, 2 * B], F32, name="psS", tag="mm")
        for k in range(8):
            nc.tensor.matmul(psS[:], ones_c[:, :], xpk[:, 2 * B * k:2 * B * (k + 1)],
                             start=(k == 0), stop=(k == 7))
        mu_r = ap.tile([1, B], F32, name="mu_r", tag="mu_r")
        nc.vector.tensor_scalar_mul(mu_r[:], psS[:, 0:B], 1.0 / STEPS)
        ex2 = ap.tile([1, B], F32, name="ex2", tag="ex2")
        nc.vector.tensor_scalar_mul(ex2[:], psS[:, B:2 * B], 1.0 / STEPS)
        m2 = ap.tile([1, B], F32, name="m2", tag="m2")
        nc.vector.tensor_mul(m2[:], mu_r[:], mu_r[:])
        var = ap.tile([1, B], F32, name="var", tag="var")
        nc.vector.tensor_sub(var[:], ex2[:], m2[:])
        vpe = ap.tile([1, B], F32, name="vpe", tag="vpe")
        nc.vector.tensor_scalar_add(vpe[:], var[:], 1e-5)
        lnv = ap.tile([1, B], F32, name="lnv", tag="lnv")
        nc.scalar.activation(lnv[:], vpe[:], AF.Ln)
        inv_r = ap.tile([1, B], F32, name="inv_r", tag="inv_r")
        nc.scalar.activation(inv_r[:], lnv[:], AF.Exp, scale=-0.5)
        sd_r = ap.tile([1, B], F32, name="sd_r", tag="sd_r")
        nc.vector.tensor_mul(sd_r[:], vpe[:], inv_r[:])
        nmu_r = ap.tile([1, B], F32, name="nmu_r", tag="nmu_r")
        nc.vector.tensor_scalar_mul(nmu_r[:], mu_r[:], -1.0)
        nmu_b = ap.tile([1, B], BF16, name="nmu_b", tag="nmu_b")
        nc.vector.tensor_copy(nmu_b[:], nmu_r[:])
        sd_b = ap.tile([1, B], BF16, name="sd_b", tag="sd_b")
        nc.vector.tensor_copy(sd_b[:], sd_r[:])

        # invT [b, 1] f32 silu scale, via bf16 hi/lo PE transposes
        ivh = ap.tile([1, B], BF16, name="ivh", tag="ivh")
        nc.vector.tensor_copy(ivh[:], inv_r[:])
        ivl = ap.tile([1, B], BF16, name="ivl", tag="ivl")
        nc.vector.tensor_sub(ivl[:], inv_r[:], ivh[:])
        psITh = ptr.tile([128, 1], BF16, name="psITh", tag="tr")
        nc.tensor.transpose(psITh[:], ivh[:], ident_b[0:1, 0:1])
        psITl = ptr.tile([128, 1], BF16, name="psITl", tag="tr")
        nc.tensor.transpose(psITl[:], ivl[:], ident_b[0:1, 0:1])
        ivhc = ap.tile([128, 1], BF16, name="ivhc", tag="ivhc")
        nc.vector.tensor_copy(ivhc[:], psITh[:])
        invT = ap.tile([128, 1], F32, name="invT", tag="invT")
        nc.vector.tensor_add(invT[:], ivhc[:], psITl[:])

        # ---- stage 1: full xz in 4 chunks of 1024 cols; u, gate, P ----
        # chunk c covers Wi' cols [1024c, 1024c+1024): c=0,1 xi; c=2,3 res.
        # order (0, 2, 1, 3) so each xi/res pair multiplies into P and frees.
        Pb = ap.tile([128, E], BF16, name="Pb", tag="Pb")
        uT_sb = ap.tile([128, 16 * B], BF16, name="uT_sb", tag="stA")
        acts = {}
        for c in (0, 2, 1, 3):
            psXZ = pmm.tile([128, 1024], F32, name=f"psXZ{c}", tag="mm")
            for h in range(2):
                lo = 1024 * c + 512 * h
                nc.tensor.matmul(psXZ[:, 512 * h:512 * (h + 1)], nmu_b[:],
                                 seedR[:, lo:lo + 512], start=True, stop=False)
                nc.tensor.matmul(psXZ[:, 512 * h:512 * (h + 1)], sd_b[:],
                                 seedR[:, 4096 + lo:4096 + lo + 512],
                                 start=False, stop=False)
                for k in range(8):
                    nc.tensor.matmul(
                        psXZ[:, 512 * h:512 * (h + 1)],
                        xpk[:, 2 * B * k:2 * B * k + B],
                        wi[:, 2 * E * k + lo:2 * E * k + lo + 512],
                        start=False, stop=(k == 7))
            act = apc.tile([128, 1024], BF16, name=f"act{c}", tag="uact")
            nc.scalar.activation(act[:], psXZ[:], AF.Silu, scale=invT[:, 0:1])
            acts[c] = act
            if c < 2:
                for j in range(8):
                    psT = ptr.tile([128, 128], BF16, name=f"psU{c}_{j}", tag="tr")
                    nc.tensor.transpose(psT[:], act[:, 128 * j:128 * (j + 1)],
                                        ident_b[:, :])
                    dst = uT_sb[:, 1024 * c + 128 * j:1024 * c + 128 * (j + 1)]
                    if j % 2 == 0:
                        nc.vector.tensor_copy(dst, psT[:])
                    else:
                        nc.scalar.copy(dst, psT[:])
            else:
                nc.vector.tensor_mul(Pb[:, 1024 * (c - 2):1024 * (c - 1)],
                                     acts[c - 2][:], act[:])

        # ---- x_dbl: d_r [b,512] and Bm|Cm [b,1024], full ----
        psDR = pmm.tile([128, DTR], F32, name="psDR", tag="mm")
        for k in range(16):
            nc.tensor.matmul(psDR[:], uT_sb[:, B * k:B * (k + 1)],
                             wx[:, XW * k:XW * k + DTR],
                             start=(k == 0), stop=(k == 15))
        psBC = pmm.tile([128, 2 * N], F32, name="psBC", tag="mm")
        for k in range(16):
            for h in range(2):
                nc.tensor.matmul(psBC[:, 512 * h:512 * (h + 1)],
                                 uT_sb[:, B * k:B * (k + 1)],
                                 wx[:, XW * k + DTR + 512 * h:
                                    XW * k + DTR + 512 * (h + 1)],
                                 start=(k == 0), stop=(k == 15))
        drb = ap.tile([128, DTR], BF16, name="drb", tag="drb")
        nc.vector.tensor_copy(drb[:], psDR[:])
        bc_sb = ap.tile([128, 2 * N], BF16, name="bc_sb", tag="bc_sb")
        nc.scalar.copy(bc_sb[:], psBC[:])
        sprod = ap.tile([128, N], BF16, name="sprod", tag="sprod")
        nc.vector.tensor_mul(sprod[:], bc_sb[:, 0:N], bc_sb[:, N:2 * N])
        s_p = ap.tile([128, 1], F32, name="s_p", tag="s_p")
        nc.vector.tensor_reduce(s_p[:], sprod[:], AX.X, ALU.add)

        # d_r transposes -> [dr-tile, b] stationaries
        drT = ap.tile([128, 4 * B], BF16, name="drT", tag="drT")
        for j in range(4):
            psT = ptr.tile([128, 128], BF16, name=f"psD{j}", tag="tr")
            nc.tensor.transpose(psT[:], drb[:, 128 * j:128 * (j + 1)], ident_b[:, :])
            if j % 2 == 0:
                nc.vector.tensor_copy(drT[:, 128 * j:128 * (j + 1)], psT[:])
            else:
                nc.scalar.copy(drT[:, 128 * j:128 * (j + 1)], psT[:])

        # ---- delta (full, 2 chunks); y1s = s*delta*P ; Pb ----
        y1b = ap.tile([128, E], BF16, name="y1b", tag="y1b")
        for c in range(2):
            psD = pmm.tile([128, 1024], F32, name=f"psDl{c}", tag="mm")
            for h in range(2):
                lo = 1024 * c + 512 * h
                nc.tensor.matmul(psD[:, 512 * h:512 * (h + 1)], ones2[:, :],
                                 sdt[:, lo:lo + 512], start=True, stop=False)
                for j in range(4):
                    nc.tensor.matmul(
                        psD[:, 512 * h:512 * (h + 1)],
                        drT[:, B * j:B * (j + 1)],
                        wdt[:, E * j + lo:E * j + lo + 512],
                        start=False, stop=(j == 3))
            exd = apd.tile([128, 1024], F32, name=f"exd{c}", tag="dact")
            nc.scalar.activation(exd[:], psD[:], AF.Exp)
            dl = apd.tile([128, 1024], BF16, name=f"dl{c}", tag="dact3")
            nc.scalar.activation(dl[:], exd[:], AF.Ln, bias=1.0)
            ds = apd.tile([128, 1024], F32, name=f"ds{c}", tag="dact")
            nc.vector.tensor_scalar_mul(ds[:], dl[:], s_p[:, 0:1])
            ds2 = apd.tile([128, 1024], BF16, name=f"ds2{c}", tag="dact2")
            nc.vector.tensor_add(ds2[:], ds[:], dbar[:, 1024 * c:1024 * (c + 1)])
            nc.vector.tensor_mul(y1b[:, 1024 * c:1024 * (c + 1)], ds2[:],
                                 Pb[:, 1024 * c:1024 * (c + 1)])

        # ---- tail psum: hb_d seed + x @ Wd + FiLM gain (emitted late so the
        # PE never stalls on the late-arriving Wd/Wfg weights) ----
        psQ = pax.tile([128, ZS], F32, name="psQ", tag="paxq")
        nc.tensor.matmul(psQ[:], ones2[:, :], shd[:, :], start=True, stop=False)
        for k in range(8):
            nc.tensor.matmul(psQ[:], xpk[:, 2 * B * k:2 * B * k + B],
                             wd[:, ZS * k:ZS * (k + 1)], start=False, stop=False)
        psG = pax.tile([128, ZS], F32, name="psG", tag="paxg")
        nc.tensor.matmul(psG[:], ones2[:, :], sfg[:, :], start=True, stop=False)
        for k in range(4):
            nc.tensor.matmul(psG[:], cpk[:, B * k:B * (k + 1)],
                             wfg[:, ZS * k:ZS * (k + 1)], start=False, stop=(k == 3))
        g_sb = ap.tile([128, ZS], F32, name="g_sb", tag="g_sb")
        nc.vector.tensor_copy(g_sb[:], psG[:])

        # transposes of y1s and P -> stationaries for the q contraction
        y1T = ap.tile([128, 16 * B], BF16, name="y1T", tag="stA")
        for j in range(16):
            psT = ptr.tile([128, 128], BF16, name=f"psY{j}", tag="tr")
            nc.tensor.transpose(psT[:], y1b[:, 128 * j:128 * (j + 1)], ident_b[:, :])
            if j % 2 == 0:
                nc.vector.tensor_copy(y1T[:, 128 * j:128 * (j + 1)], psT[:])
            else:
                nc.scalar.copy(y1T[:, 128 * j:128 * (j + 1)], psT[:])

        # ---- finish the tail psum: + y_comb @ Wod ; gelu ; out ----
        for k in range(16):
            nc.tensor.matmul(psQ[:], y1T[:, B * k:B * (k + 1)],
                             wod[:, ZS * k:ZS * (k + 1)],
                             start=False, stop=(k == 15))
        z = ap.tile([128, ZS], F32, name="z", tag="z")
        nc.scalar.activation(z[:], psQ[:], AF.Gelu)
        zf = ap.tile([128, ZS], BF16, name="zf", tag="zf")
        nc.vector.tensor_mul(zf[:], z[:], g_sb[:])
        psZT = ptr.tile([ZS, 128], BF16, name="psZT", tag="tr")
        nc.tensor.transpose(psZT[:], zf[:], ident_b[:, :])
        zfT = ap.tile([ZS, B], BF16, name="zfT", tag="zfT")
        nc.vector.tensor_copy(zfT[:], psZT[:])

        out_sb = ap.tile([128, STEPS], BF16, name="out_sb", tag="out_sb")
        for m in range(2):
            psO = pmm.tile([128, 512], F32, name=f"psO{m}", tag="mm")
            nc.tensor.matmul(psO[:], zfT[:, :], wo[:, 512 * m:512 * (m + 1)],
                             start=True, stop=False)
            nc.tensor.matmul(psO[:], cts[:, :], wfbo[:, 512 * m:512 * (m + 1)],
                             start=False, stop=True)
            if m == 0:
                nc.vector.tensor_copy(out_sb[:, 512 * m:512 * (m + 1)], psO[:])
            else:
                nc.scalar.copy(out_sb[:, 512 * m:512 * (m + 1)], psO[:])
            nc.sync.dma_start(out_d[:, 512 * m:512 * (m + 1)],
                              out_sb[:, 512 * m:512 * (m + 1)])

    return nc


_CACHE = {}


def _get_nc() -> bass.Bass:
    if "nc" not in _CACHE:
        _CACHE["nc"] = build_nc()
    return _CACHE["nc"]


def _blk(t, K):
    # [K*128, M] row-major -> [128, K*M] so each partition's DMA is contiguous
    M = t.shape[1]
    return np.ascontiguousarray(
        t.reshape(K, 128, M).transpose(1, 0, 2).reshape(128, K * M))


def _hilo(a):
    hi = a.astype(BF)
    lo = (a - hi.astype(np.float32)).astype(BF)
    return hi, lo


def kernel(**inputs) -> np.ndarray:
    inp = {k: np.asarray(v) for k, v in inputs.items()}
    f32 = np.float32
    x = inp["x"].reshape(B, STEPS).astype(f32)
    c = inp["c"].astype(f32)
    ln_g = inp["ln_g"].astype(f32)
    ln_b = inp["ln_b"].astype(f32)
    W_in = inp["W_in"].astype(f32)
    conv_w = inp["conv_w"].astype(f32)
    conv_b = inp["conv_b"].astype(f32)
    W_x = inp["W_x"].astype(f32)
    W_dt = inp["W_dt"].astype(f32)
    b_dt = inp["b_dt"].astype(f32)
    D = inp["D"].astype(f32)
    W_out = inp["W_out"].astype(f32)
    b_out = inp["b_out"].astype(f32)
    W_d = inp["W_d"].astype(f32)
    b_d = inp["b_d"].astype(f32)
    W_f = inp["W_f"].astype(f32)
    b_f = inp["b_f"].astype(f32)
    W_o = inp["W_o"].astype(f32)
    b_o = inp["b_o"].astype(f32)

    # host constant folding (weight-only)
    cw = conv_w[3, 0, :]
    Wi_full = np.concatenate([(ln_g[:, None] * W_in[:, :E]) * cw[None, :],
                              ln_g[:, None] * W_in[:, E:]], axis=1)  # (1024,4096)
    bias_xz = ln_b @ W_in
    bias_k = np.concatenate([bias_xz[:E] * cw + conv_b, bias_xz[E:]])  # (4096,)
    W_od = W_out @ W_d                      # (2048, 512)
    hb_d = b_out @ W_d + b_d                # (512,)
    W_fg = W_f[:, :N]
    b_fg = b_f[:N]
    W_fbo = W_f[:, N:] @ W_o                # (512, 1024)
    hb_o = b_f[N:] @ W_o + b_o              # (1024,)

    Wi_bf = Wi_full.astype(BF)
    cs_full = Wi_bf.astype(f32).sum(axis=0)                    # colsum of bf16 Wi
    seedR = np.concatenate([cs_full, bias_k])[None, :].astype(BF)  # (1, 8192)
    dh, dl = _hilo(b_dt)
    sdt = np.stack([dh, dl]).astype(BF)                        # (2, 2048)

    xT_bf = np.ascontiguousarray(x.T).astype(BF)               # (1024, 128)
    xsqT_bf = np.ascontiguousarray((x * x).T).astype(BF)
    xpk = np.concatenate([xT_bf, xsqT_bf], axis=1)             # (1024, 256)
    cT_bf = np.ascontiguousarray(c.T).astype(BF)               # (512, 128)
    Wx_bf = np.ascontiguousarray(W_x).astype(BF)               # (2048, 1536)
    Wdt_bf = np.ascontiguousarray(W_dt).astype(BF)             # (512, 2048)

    in_maps = []
    for k in range(R):
        zs = slice(ZS * k, ZS * (k + 1))
        cs = slice(CS * k, CS * (k + 1))
        hh, hl = _hilo(hb_d[zs])
        gh, gl = _hilo(b_fg[zs])
        in_maps.append({
            "xpk": _blk(xpk, 8),
            "Wi": Wi_bf,
            "seedR": seedR,
            "Wx": Wx_bf,
            "Wdt": _blk(Wdt_bf, 4),
            "sdt": sdt,
            "Wod": _blk(np.ascontiguousarray(W_od[:, zs]).astype(BF), 16),
            "Drow": D[None, :].astype(BF),
            "Wd": _blk(np.ascontiguousarray(W_d[:, zs]).astype(BF), 8),
            "shd": np.stack([hh, hl]).astype(BF),
            "cT": _blk(cT_bf, 4),
            "Wfg": _blk(np.ascontiguousarray(W_fg[:, zs]).astype(BF), 4),
            "sfg": np.stack([gh, gl]).astype(BF),
            "Wo": np.ascontiguousarray(W_o[zs, :]).astype(BF),
            "Wfbo": np.ascontiguousarray(W_fbo[cs, :]).astype(BF),
            "cTs": np.ascontiguousarray(cT_bf[cs, :]),
        })

    nc = _get_nc()
    res = run_bass_kernel_spmd(nc, in_maps, core_ids=list(range(R)),
                               **_CACHE.get("run_kwargs", {}))
    _CACHE["last_results"] = res
    out = np.zeros((B, STEPS), np.float64)
    for r in res.results:
        out += r["outp"].astype(np.float64)
    out = out.astype(f32) + hb_o[None, :]
    return out.astype(f32)
